# revision 4
# baseline (speedup 1.0000x reference)
"""GCNII (8-layer) Trainium2 kernel, 8-core SPMD.

Sharding: nodes (and incoming edges) by destination across 8 cores; x
replicated per-layer via AllGather as a bf16 gather table in HBM.

Per layer, per core: dma_gather of per-edge source rows (dst-window-major
slot stream, lo/hi split for int16 index range), then one matmul per
128-slot chunk: y_T[window] += msg_chunk^T @ S_chunk, where S (streamed
from HBM, graph-static) carries the edge weights and the dst one-hot.
Dense GCNII update runs feature-major on-chip.
"""
import sys
import os
import math

sys.path.insert(0, "/opt/trn_rl_repo")

import numpy as np
import ml_dtypes

N = 50000
E = 800000
IN = 512
H = 128
C = 40
L = 8
ALPHA = 0.1
LAMDA = 0.5
R = 8                  # cores
SH = N // R            # 6250 real nodes per shard
SHP = 6272             # padded shard (49*128)
NW = SHP // 128        # 49 dst windows per core
NP = R * SHP           # 50176 padded table rows
LO_LIM = 32768
HI_BASE = NP - 32768   # 17408
CH = 128               # slots per chunk
CALL_CH = 8            # chunks per gather call (1024 idx, ring-safe)
SBLK = 16              # chunks per S-stream block

_BF16 = ml_dtypes.bfloat16


def _preprocess(edge_index, norm_A):
    """Build per-core slot streams, gather indices, and S matrices."""
    s = edge_index[0].astype(np.int64)
    d = edge_index[1].astype(np.int64)
    w = norm_A.astype(np.float32)
    trow = (s // SH) * SHP + (s % SH)          # padded table row of src
    owner = d // SH
    dl = d % SH                                 # local dst on owner core
    win = dl // 128
    half = (trow >= LO_LIM).astype(np.int64)    # 0=lo, 1=hi

    # per-core, per-(window, half) edge counts -> uniform chunk grid
    cores = []
    nch = np.zeros((NW, 2), dtype=np.int64)
    for r in range(R):
        sel = owner == r
        tr_r, dl_r, w_r, win_r, h_r = trow[sel], dl[sel], w[sel], win[sel], half[sel]
        order = np.lexsort((h_r, win_r))
        tr_r, dl_r, w_r, win_r, h_r = (
            tr_r[order], dl_r[order], w_r[order], win_r[order], h_r[order])
        key = win_r * 2 + h_r
        cnt = np.bincount(key, minlength=NW * 2).reshape(NW, 2)
        nch = np.maximum(nch, (cnt + CH - 1) // CH)
        cores.append((tr_r, dl_r, w_r, key, cnt))

    # global chunk layout: window-major, lo then hi; chunk_base[w,h] in chunks
    flat = nch.reshape(-1)
    chunk_base = np.concatenate([[0], np.cumsum(flat)])[:-1].reshape(NW, 2)
    nchunk = int(flat.sum())
    nchunk_pad = ((nchunk + SBLK - 1) // SBLK) * SBLK
    nslot = nchunk * CH

    # gather calls: per (w,h) group, runs of <= CALL_CH chunks
    calls = []  # (chunk0, nchunks, half)
    for wv in range(NW):
        for h in range(2):
            c0, n = int(chunk_base[wv, h]), int(nch[wv, h])
            while n > 0:
                take = min(CALL_CH, n)
                calls.append((c0, take, h))
                c0 += take
                n -= take

    # per-window matmul chunk ranges
    win_chunks = []  # (first_chunk, nchunks_total) per window
    for wv in range(NW):
        first = int(chunk_base[wv, 0])
        total = int(nch[wv, 0] + nch[wv, 1])
        win_chunks.append((first, total))

    per_core = []
    for r in range(R):
        tr_r, dl_r, w_r, key, cnt = cores[r]
        # slot index of each edge: chunk_base[key]*CH + rank within group
        starts = np.concatenate([[0], np.cumsum(cnt.reshape(-1))])[:-1]
        rank = np.arange(tr_r.size) - starts[key]
        slot = chunk_base.reshape(-1)[key] * CH + rank
        chunk_of = slot // CH
        k_of = slot % CH
        m_of = dl_r % 128
        idx16 = np.where(tr_r < LO_LIM, tr_r, tr_r - HI_BASE).astype(np.int16)

        idx_arr = np.zeros(nslot, dtype=np.int16)
        idx_arr[slot] = idx16
        # wrapped [128, nslot/16] layout, 8x replicated over partition groups
        idx_w = np.tile(idx_arr.reshape(-1, 16).T, (8, 1)).copy()

        S = np.zeros((nchunk_pad, CH, 128), dtype=np.float32)
        S[chunk_of, k_of, m_of] = w_r
        # k-major within each 16-chunk block: [blk, k, c, m]
        Sb = np.ascontiguousarray(
            S.reshape(nchunk_pad // SBLK, SBLK, CH, 128).transpose(0, 2, 1, 3)
        ).astype(_BF16)
        per_core.append({"idx": idx_w, "S": Sb})

    meta = {
        "nchunk": nchunk,
        "nchunk_pad": nchunk_pad,
        "nslot": nslot,
        "calls": calls,
        "win_chunks": win_chunks,
    }
    return per_core, meta


_CACHE = {}


def _build(meta):
    key = (meta["nchunk"], tuple(meta["calls"]), tuple(meta["win_chunks"]))
    if key in _CACHE:
        return _CACHE[key]

    import concourse.bass as bass
    import concourse.mybir as mybir
    from concourse import bacc
    from concourse.tile import TileContext

    f32 = mybir.dt.float32
    bf16 = mybir.dt.bfloat16
    i16 = mybir.dt.int16
    AX = mybir.AxisListType
    OP = mybir.AluOpType
    ACTF = mybir.ActivationFunctionType

    nchunk = meta["nchunk"]
    nchunk_pad = meta["nchunk_pad"]
    nslot = meta["nslot"]
    calls = meta["calls"]
    win_chunks = meta["win_chunks"]
    nsblk = nchunk_pad // SBLK

    nc = bacc.Bacc("TRN2", target_bir_lowering=False, debug=False,
                   num_devices=R, num_swdge_queues=4)

    feat_in = nc.dram_tensor("features", [SHP, IN], f32, kind="ExternalInput")
    wfc0_in = nc.dram_tensor("w_fc0", [IN, H], f32, kind="ExternalInput")
    bfc0_in = nc.dram_tensor("b_fc0", [H], f32, kind="ExternalInput")
    conv_in = nc.dram_tensor("conv_w", [L, H, H], f32, kind="ExternalInput")
    wfc1_in = nc.dram_tensor("w_fc1", [H, C], f32, kind="ExternalInput")
    bfc1_in = nc.dram_tensor("b_fc1", [C], f32, kind="ExternalInput")
    idx_in = nc.dram_tensor("idx", [128, nslot // 16], i16, kind="ExternalInput")
    S_in = nc.dram_tensor("S", [nsblk, 128, SBLK, 128], bf16, kind="ExternalInput")
    ident_in = nc.dram_tensor("identb", [128, 128], bf16, kind="ExternalInput")
    ident40_in = nc.dram_tensor("ident40", [C, C], f32, kind="ExternalInput")
    out_ext = nc.dram_tensor("out", [SHP, C], f32, kind="ExternalOutput")

    xsh_d = nc.dram_tensor("xsh", [SHP, H], bf16)
    xag_d = nc.dram_tensor("xag", [NP, H], bf16, addr_space="Shared")
    xag_lo = xag_d[0:LO_LIM, :]
    xag_hi = xag_d[HI_BASE:NP, :]

    with TileContext(nc) as tc:
        with (
            tc.tile_pool(name="persist", bufs=1) as pp,
            tc.tile_pool(name="msg", bufs=6) as mp,
            tc.tile_pool(name="sstream", bufs=3) as sp,
            tc.tile_pool(name="work", bufs=2) as wkp,
            tc.tile_pool(name="psy", bufs=3, space="PSUM") as psy,
            tc.tile_pool(name="psz", bufs=2, space="PSUM") as psz,
            tc.tile_pool(name="pst", bufs=2, space="PSUM") as pst,
        ):
            # ---- persistent tiles
            idx_t = pp.tile([128, nslot // 16], i16)
            nc.sync.dma_start(idx_t[:], idx_in[:])
            identb = pp.tile([128, 128], bf16)
            nc.sync.dma_start(identb[:], ident_in[:])
            ident40 = pp.tile([C, C], f32)
            nc.sync.dma_start(ident40[:], ident40_in[:])
            wfc0_f = pp.tile([128, 4, H], f32)
            for j in range(4):
                nc.sync.dma_start(wfc0_f[:, j, :], wfc0_in[j * 128:(j + 1) * 128, :])
            wfc0_t = pp.tile([128, 4, H], bf16)
            nc.vector.tensor_copy(wfc0_t[:], wfc0_f[:])
            bfc0_t = pp.tile([H, 1], f32)
            nc.sync.dma_start(bfc0_t[:], bfc0_in[:, None])
            conv_f = pp.tile([128, L, H], f32)
            for j in range(L):
                nc.sync.dma_start(conv_f[:, j, :], conv_in[j])
            conv_t = pp.tile([128, L, H], bf16)
            nc.vector.tensor_copy(conv_t[:], conv_f[:])
            wfc1_f = pp.tile([H, C], f32)
            nc.sync.dma_start(wfc1_f[:], wfc1_in[:])
            wfc1_t = pp.tile([H, C], bf16)
            nc.vector.tensor_copy(wfc1_t[:], wfc1_f[:])
            bfc1_t = pp.tile([C, 1], f32)
            nc.sync.dma_start(bfc1_t[:], bfc1_in[:, None])

            x_T = pp.tile([128, SHP], bf16)       # current x, feature-major
            h0a = pp.tile([128, SHP], bf16)       # ALPHA * x0
            sup_T = pp.tile([128, SHP], bf16)     # support, feature-major
            stage = pp.tile([128, NW, 128], bf16)  # node-major x staging

            # ---- fc0: x0 = relu(features @ w_fc0 + b_fc0), feature-major
            for i in range(NW):
                ftf = wkp.tile([128, IN], f32, tag="ftf")
                nc.sync.dma_start(ftf[:], feat_in[i * 128:(i + 1) * 128, :])
                ft = wkp.tile([128, IN], bf16, tag="ft")
                nc.vector.tensor_copy(ft[:], ftf[:])
                ftT = wkp.tile([128, 4, 128], bf16, tag="ftT")
                for j in range(4):
                    tp = pst.tile([128, 128], bf16, tag="tp")
                    nc.tensor.matmul(tp[:], ft[:, j * 128:(j + 1) * 128], identb[:],
                                     is_transpose=True)
                    nc.scalar.copy(ftT[:, j, :], tp[:])
                x0p = psy.tile([128, 128], f32, tag="yw")
                for j in range(4):
                    nc.tensor.matmul(x0p[:], wfc0_t[:, j, :], ftT[:, j, :],
                                     start=(j == 0), stop=(j == 3))
                nc.scalar.activation(x_T[:, i * 128:(i + 1) * 128], x0p[:],
                                     ACTF.Relu, bias=bfc0_t[:], scale=1.0)
            nc.vector.tensor_scalar_mul(h0a[:], x_T[:], ALPHA)

            def export_x():
                # transpose x_T -> node-major staging -> xsh -> AllGather
                for i in range(NW):
                    tp = pst.tile([128, 128], bf16, tag="tp")
                    nc.tensor.matmul(tp[:], x_T[:, i * 128:(i + 1) * 128],
                                     identb[:], is_transpose=True)
                    nc.scalar.copy(stage[:, i, :], tp[:])
                nc.sync.dma_start(xsh_d.ap().rearrange("(i p) h -> p i h", p=128),
                                  stage[:])
                nc.gpsimd.collective_compute(
                    "AllGather", OP.bypass,
                    replica_groups=[list(range(R))],
                    ins=[xsh_d[:]], outs=[xag_d[:]],
                )

            export_x()

            # ---- layers
            for l in range(1, L + 1):
                beta = float(math.log(LAMDA / l + 1.0))

                # gather all slots for this layer
                msg_tiles = {}
                for (c0, ncall, h) in calls:
                    m = mp.tile([128, CALL_CH, H], bf16, tag="msg")
                    src_ap = xag_lo if h == 0 else xag_hi
                    nidx = ncall * CH
                    nc.gpsimd.dma_gather(
                        m[:, 0:ncall, :], src_ap,
                        idx_t[:, c0 * 8:(c0 + ncall) * 8],
                        nidx, nidx, H,
                        queue_num=(len(msg_tiles) % 4),
                    )
                    for cc in range(ncall):
                        msg_tiles[c0 + cc] = (m, cc)

                # S-stream + per-window accumulation
                s_tiles = {}
                for b in range(nsblk):
                    st = sp.tile([128, SBLK, 128], bf16, tag="sstream")
                    nc.sync.dma_start(st[:], S_in[b])
                    s_tiles[b] = st

                for wv in range(NW):
                    first, total = win_chunks[wv]
                    ws = slice(wv * 128, (wv + 1) * 128)
                    if total == 0:
                        nc.vector.tensor_copy(sup_T[:, ws], h0a[:, ws])
                        continue
                    yp = psy.tile([128, 128], f32, tag="yw")
                    for t in range(total):
                        c = first + t
                        m, cc = msg_tiles[c]
                        st = s_tiles[c // SBLK]
                        nc.tensor.matmul(yp[:], m[:, cc, :], st[:, c % SBLK, :],
                                         start=(t == 0), stop=(t == total - 1))
                    # support = (1-ALPHA)*y + ALPHA*h0
                    nc.vector.scalar_tensor_tensor(
                        sup_T[:, ws], yp[:], 1.0 - ALPHA, h0a[:, ws],
                        OP.mult, OP.add)

                # dense: x = relu(beta*(sup @ W) + (1-beta)*sup)
                for i in range(13):
                    n0 = i * 512
                    n1 = min(SHP, n0 + 512)
                    zs = psz.tile([128, 512], f32, tag="z")
                    nc.tensor.matmul(zs[:, 0:n1 - n0], conv_t[:, l - 1, :],
                                     sup_T[:, n0:n1])
                    tmp = wkp.tile([128, 512], f32, tag="tmp")
                    nc.vector.scalar_tensor_tensor(
                        tmp[:, 0:n1 - n0], zs[:, 0:n1 - n0],
                        beta / (1.0 - beta), sup_T[:, n0:n1],
                        OP.mult, OP.add)
                    nc.scalar.activation(x_T[:, n0:n1], tmp[:, 0:n1 - n0],
                                         ACTF.Relu, scale=1.0 - beta)

                if l < L:
                    export_x()

            # ---- fc1: out = x8 @ w_fc1 + b_fc1, node-major
            ostage = pp.tile([128, NW, C], f32)
            osb = wkp.tile([C, 512], f32, tag="osb")
            for i in range(13):
                n0 = i * 512
                n1 = min(SHP, n0 + 512)
                op_ = psz.tile([C, 512], f32, tag="z")
                nc.tensor.matmul(op_[:, 0:n1 - n0], wfc1_t[:], x_T[:, n0:n1])
                nc.vector.tensor_scalar_add(osb[:, 0:n1 - n0], op_[:, 0:n1 - n0],
                                            bfc1_t[:])
                for k in range(n0 // 128, n1 // 128):
                    tp = pst.tile([128, C], f32, tag="tp")
                    nc.tensor.matmul(tp[:], osb[:, k * 128 - n0:(k + 1) * 128 - n0],
                                     ident40[:], is_transpose=True)
                    nc.scalar.copy(ostage[:, k, :], tp[:])
            nc.sync.dma_start(out_ext.ap().rearrange("(i p) c -> p i c", p=128),
                              ostage[:])

    nc.compile()
    _CACHE[key] = nc
    return nc


def kernel(**inputs):
    feats = np.asarray(inputs["features"], dtype=np.float32)
    edge_index = np.asarray(inputs["edge_index"])
    norm_A = np.asarray(inputs["norm_A"], dtype=np.float32)

    per_core, meta = _preprocess(edge_index, norm_A)
    nc = _build(meta)

    ident = np.eye(128, dtype=_BF16)
    ident40 = np.eye(C, dtype=np.float32)
    featp = np.zeros((SHP, IN), dtype=np.float32)

    in_maps = []
    for r in range(R):
        fp = np.zeros((SHP, IN), dtype=np.float32)
        fp[:SH] = feats[r * SH:(r + 1) * SH]
        in_maps.append({
            "features": fp,
            "w_fc0": np.asarray(inputs["w_fc0"], dtype=np.float32),
            "b_fc0": np.asarray(inputs["b_fc0"], dtype=np.float32),
            "conv_w": np.asarray(inputs["conv_w"], dtype=np.float32),
            "w_fc1": np.asarray(inputs["w_fc1"], dtype=np.float32),
            "b_fc1": np.asarray(inputs["b_fc1"], dtype=np.float32),
            "idx": per_core[r]["idx"],
            "S": per_core[r]["S"],
            "identb": ident,
            "ident40": ident40,
        })

    from concourse.bass_utils import run_bass_kernel_spmd
    trace = bool(os.environ.get("GCN_TRACE"))
    res = run_bass_kernel_spmd(nc, in_maps, list(range(R)), trace=trace)
    if trace:
        kernel.last_exec_time_ns = res.exec_time_ns
    out = np.concatenate([res.results[r]["out"][:SH] for r in range(R)], axis=0)
    return out.astype(np.float32)


# revision 5
# speedup vs baseline: 1.2203x; 1.2203x over previous
"""GCNII (8-layer) Trainium2 kernel, 8-core SPMD.

Sharding: nodes (and incoming edges) by destination across 8 cores; x
replicated per-layer via AllGather as a bf16 gather table in HBM.

Per layer, per core: dma_gather of per-edge source rows (dst-window-major
slot stream, lo/hi split for int16 index range), then one matmul per
128-slot chunk: y_T[window] += msg_chunk^T @ S_chunk, where S (streamed
from HBM, graph-static) carries the edge weights and the dst one-hot.
Dense GCNII update runs feature-major on-chip.
"""
import sys
import os
import math

sys.path.insert(0, "/opt/trn_rl_repo")

import numpy as np
import ml_dtypes

N = 50000
E = 800000
IN = 512
H = 128
C = 40
L = 8
ALPHA = 0.1
LAMDA = 0.5
R = 8                  # cores
SH = N // R            # 6250 real nodes per shard
SHP = 6272             # padded shard (49*128)
NW = SHP // 128        # 49 dst windows per core
NP = R * SHP           # 50176 padded table rows
LO_LIM = 32768
HI_BASE = NP - 32768   # 17408
CH = 128               # slots per chunk
CALL_CH = 8            # chunks per gather call (1024 idx, ring-safe)
SBLK = 16              # chunks per S-stream block

_BF16 = ml_dtypes.bfloat16


def _preprocess(edge_index, norm_A):
    """Build per-core slot streams, gather indices, and S matrices."""
    s = edge_index[0].astype(np.int64)
    d = edge_index[1].astype(np.int64)
    w = norm_A.astype(np.float32)
    trow = (s // SH) * SHP + (s % SH)          # padded table row of src
    owner = d // SH
    dl = d % SH                                 # local dst on owner core
    win = dl // 128
    half = (trow >= LO_LIM).astype(np.int64)    # 0=lo, 1=hi

    # per-core, per-(window, half) edge counts -> uniform chunk grid
    cores = []
    nch = np.zeros((NW, 2), dtype=np.int64)
    for r in range(R):
        sel = owner == r
        tr_r, dl_r, w_r, win_r, h_r = trow[sel], dl[sel], w[sel], win[sel], half[sel]
        order = np.lexsort((h_r, win_r))
        tr_r, dl_r, w_r, win_r, h_r = (
            tr_r[order], dl_r[order], w_r[order], win_r[order], h_r[order])
        key = win_r * 2 + h_r
        cnt = np.bincount(key, minlength=NW * 2).reshape(NW, 2)
        nch = np.maximum(nch, (cnt + CH - 1) // CH)
        cores.append((tr_r, dl_r, w_r, key, cnt))

    # global chunk layout: window-major, lo then hi; chunk_base[w,h] in chunks
    flat = nch.reshape(-1)
    chunk_base = np.concatenate([[0], np.cumsum(flat)])[:-1].reshape(NW, 2)
    nchunk = int(flat.sum())
    nchunk_pad = ((nchunk + SBLK - 1) // SBLK) * SBLK
    nslot = nchunk * CH

    # gather calls: per (w,h) group, runs of <= CALL_CH chunks
    calls = []  # (chunk0, nchunks, half)
    for wv in range(NW):
        for h in range(2):
            c0, n = int(chunk_base[wv, h]), int(nch[wv, h])
            while n > 0:
                take = min(CALL_CH, n)
                calls.append((c0, take, h))
                c0 += take
                n -= take

    # per-window matmul chunk ranges
    win_chunks = []  # (first_chunk, nchunks_total) per window
    for wv in range(NW):
        first = int(chunk_base[wv, 0])
        total = int(nch[wv, 0] + nch[wv, 1])
        win_chunks.append((first, total))

    per_core = []
    for r in range(R):
        tr_r, dl_r, w_r, key, cnt = cores[r]
        # slot index of each edge: chunk_base[key]*CH + rank within group
        starts = np.concatenate([[0], np.cumsum(cnt.reshape(-1))])[:-1]
        rank = np.arange(tr_r.size) - starts[key]
        slot = chunk_base.reshape(-1)[key] * CH + rank
        chunk_of = slot // CH
        k_of = slot % CH
        m_of = dl_r % 128
        idx16 = np.where(tr_r < LO_LIM, tr_r, tr_r - HI_BASE).astype(np.int16)

        idx_arr = np.zeros(nslot, dtype=np.int16)
        idx_arr[slot] = idx16
        # wrapped [128, nslot/16] layout, 8x replicated over partition groups
        idx_w = np.tile(idx_arr.reshape(-1, 16).T, (8, 1)).copy()

        S = np.zeros((nchunk_pad, CH, 128), dtype=np.float32)
        S[chunk_of, k_of, m_of] = w_r
        # k-major within each 16-chunk block: [blk, k, c, m]
        Sb = np.ascontiguousarray(
            S.reshape(nchunk_pad // SBLK, SBLK, CH, 128).transpose(0, 2, 1, 3)
        ).astype(_BF16)
        per_core.append({"idx": idx_w, "S": Sb})

    meta = {
        "nchunk": nchunk,
        "nchunk_pad": nchunk_pad,
        "nslot": nslot,
        "calls": calls,
        "win_chunks": win_chunks,
    }
    return per_core, meta


_CACHE = {}


def _build(meta):
    key = (meta["nchunk"], tuple(meta["calls"]), tuple(meta["win_chunks"]))
    if key in _CACHE:
        return _CACHE[key]

    import concourse.bass as bass
    import concourse.mybir as mybir
    from concourse import bacc
    from concourse.tile import TileContext

    f32 = mybir.dt.float32
    bf16 = mybir.dt.bfloat16
    i16 = mybir.dt.int16
    AX = mybir.AxisListType
    OP = mybir.AluOpType
    ACTF = mybir.ActivationFunctionType

    nchunk = meta["nchunk"]
    nchunk_pad = meta["nchunk_pad"]
    nslot = meta["nslot"]
    calls = meta["calls"]
    win_chunks = meta["win_chunks"]
    nsblk = nchunk_pad // SBLK

    nc = bacc.Bacc("TRN2", target_bir_lowering=False, debug=False,
                   num_devices=R, num_swdge_queues=4)

    feat_in = nc.dram_tensor("features", [SHP, IN], f32, kind="ExternalInput")
    wfc0_in = nc.dram_tensor("w_fc0", [IN, H], f32, kind="ExternalInput")
    bfc0_in = nc.dram_tensor("b_fc0", [H], f32, kind="ExternalInput")
    conv_in = nc.dram_tensor("conv_w", [L, H, H], f32, kind="ExternalInput")
    wfc1_in = nc.dram_tensor("w_fc1", [H, C], f32, kind="ExternalInput")
    bfc1_in = nc.dram_tensor("b_fc1", [C], f32, kind="ExternalInput")
    idx_in = nc.dram_tensor("idx", [128, nslot // 16], i16, kind="ExternalInput")
    S_in = nc.dram_tensor("S", [nsblk, 128, SBLK, 128], bf16, kind="ExternalInput")
    ident_in = nc.dram_tensor("identb", [128, 128], bf16, kind="ExternalInput")
    ident40_in = nc.dram_tensor("ident40", [C, C], f32, kind="ExternalInput")
    out_ext = nc.dram_tensor("out", [SHP, C], f32, kind="ExternalOutput")

    xsh_d = nc.dram_tensor("xsh", [SHP, H], bf16)
    xag_d = nc.dram_tensor("xag", [NP, H], bf16, addr_space="Shared")
    xag_lo = xag_d[0:LO_LIM, :]
    xag_hi = xag_d[HI_BASE:NP, :]

    with TileContext(nc) as tc:
        with (
            tc.tile_pool(name="persist", bufs=1) as pp,
            tc.tile_pool(name="msg", bufs=12) as mp,
            tc.tile_pool(name="sstream", bufs=5) as sp,
            tc.tile_pool(name="work", bufs=2) as wkp,
            tc.tile_pool(name="psy", bufs=3, space="PSUM") as psy,
            tc.tile_pool(name="psz", bufs=2, space="PSUM") as psz,
            tc.tile_pool(name="pst", bufs=2, space="PSUM") as pst,
        ):
            # ---- persistent tiles
            idx_t = pp.tile([128, nslot // 16], i16)
            nc.sync.dma_start(idx_t[:], idx_in[:])
            identb = pp.tile([128, 128], bf16)
            nc.sync.dma_start(identb[:], ident_in[:])
            ident40 = pp.tile([C, C], f32)
            nc.sync.dma_start(ident40[:], ident40_in[:])
            wfc0_f = pp.tile([128, 4, H], f32)
            for j in range(4):
                nc.sync.dma_start(wfc0_f[:, j, :], wfc0_in[j * 128:(j + 1) * 128, :])
            wfc0_t = pp.tile([128, 4, H], bf16)
            nc.vector.tensor_copy(wfc0_t[:], wfc0_f[:])
            bfc0_t = pp.tile([H, 1], f32)
            nc.sync.dma_start(bfc0_t[:], bfc0_in[:, None])
            conv_f = pp.tile([128, L, H], f32)
            for j in range(L):
                nc.sync.dma_start(conv_f[:, j, :], conv_in[j])
            conv_t = pp.tile([128, L, H], bf16)
            nc.vector.tensor_copy(conv_t[:], conv_f[:])
            wfc1_f = pp.tile([H, C], f32)
            nc.sync.dma_start(wfc1_f[:], wfc1_in[:])
            wfc1_t = pp.tile([H, C], bf16)
            nc.vector.tensor_copy(wfc1_t[:], wfc1_f[:])
            bfc1_t = pp.tile([C, 1], f32)
            nc.sync.dma_start(bfc1_t[:], bfc1_in[:, None])

            x_T = pp.tile([128, SHP], bf16)       # current x, feature-major
            h0a = pp.tile([128, SHP], bf16)       # ALPHA * x0
            sup_T = pp.tile([128, SHP], bf16)     # support, feature-major
            stage = pp.tile([128, NW, 128], bf16)  # node-major x staging

            # ---- fc0: x0 = relu(features @ w_fc0 + b_fc0), feature-major
            for i in range(NW):
                ftf = wkp.tile([128, IN], f32, tag="ftf")
                nc.sync.dma_start(ftf[:], feat_in[i * 128:(i + 1) * 128, :])
                ft = wkp.tile([128, IN], bf16, tag="ft")
                nc.vector.tensor_copy(ft[:], ftf[:])
                ftT = wkp.tile([128, 4, 128], bf16, tag="ftT")
                for j in range(4):
                    tp = pst.tile([128, 128], bf16, tag="tp")
                    nc.tensor.matmul(tp[:], ft[:, j * 128:(j + 1) * 128], identb[:],
                                     is_transpose=True)
                    nc.scalar.copy(ftT[:, j, :], tp[:])
                x0p = psy.tile([128, 128], f32, tag="yw")
                for j in range(4):
                    nc.tensor.matmul(x0p[:], wfc0_t[:, j, :], ftT[:, j, :],
                                     start=(j == 0), stop=(j == 3))
                nc.scalar.activation(x_T[:, i * 128:(i + 1) * 128], x0p[:],
                                     ACTF.Relu, bias=bfc0_t[:], scale=1.0)
            nc.vector.tensor_scalar_mul(h0a[:], x_T[:], ALPHA)

            def export_x():
                # transpose x_T -> node-major staging -> xsh -> AllGather
                for i in range(NW):
                    tp = pst.tile([128, 128], bf16, tag="tp")
                    nc.tensor.matmul(tp[:], x_T[:, i * 128:(i + 1) * 128],
                                     identb[:], is_transpose=True)
                    nc.scalar.copy(stage[:, i, :], tp[:])
                nc.sync.dma_start(xsh_d.ap().rearrange("(i p) h -> p i h", p=128),
                                  stage[:])
                nc.gpsimd.collective_compute(
                    "AllGather", OP.bypass,
                    replica_groups=[list(range(R))],
                    ins=[xsh_d[:]], outs=[xag_d[:]],
                )

            export_x()

            # ---- layers
            for l in range(1, L + 1):
                beta = float(math.log(LAMDA / l + 1.0))

                # gather all slots for this layer
                msg_tiles = {}
                for ci, (c0, ncall, h) in enumerate(calls):
                    m = mp.tile([128, CALL_CH, H], bf16, tag="msg")
                    src_ap = xag_lo if h == 0 else xag_hi
                    nidx = ncall * CH
                    nc.gpsimd.dma_gather(
                        m[:, 0:ncall, :], src_ap,
                        idx_t[:, c0 * 8:(c0 + ncall) * 8],
                        nidx, nidx, H,
                        queue_num=(ci % 4),
                    )
                    for cc in range(ncall):
                        msg_tiles[c0 + cc] = (m, cc)

                # S-stream + per-window accumulation
                s_tiles = {}
                for b in range(nsblk):
                    st = sp.tile([128, SBLK, 128], bf16, tag="sstream")
                    nc.sync.dma_start(st[:], S_in[b])
                    s_tiles[b] = st

                for wv in range(NW):
                    first, total = win_chunks[wv]
                    ws = slice(wv * 128, (wv + 1) * 128)
                    if total == 0:
                        nc.vector.tensor_copy(sup_T[:, ws], h0a[:, ws])
                        continue
                    yp = psy.tile([128, 128], f32, tag="yw")
                    for t in range(total):
                        c = first + t
                        m, cc = msg_tiles[c]
                        st = s_tiles[c // SBLK]
                        nc.tensor.matmul(yp[:], m[:, cc, :], st[:, c % SBLK, :],
                                         start=(t == 0), stop=(t == total - 1))
                    # support = (1-ALPHA)*y + ALPHA*h0
                    nc.vector.scalar_tensor_tensor(
                        sup_T[:, ws], yp[:], 1.0 - ALPHA, h0a[:, ws],
                        OP.mult, OP.add)

                # dense: x = relu(beta*(sup @ W) + (1-beta)*sup)
                for i in range(13):
                    n0 = i * 512
                    n1 = min(SHP, n0 + 512)
                    zs = psz.tile([128, 512], f32, tag="z")
                    nc.tensor.matmul(zs[:, 0:n1 - n0], conv_t[:, l - 1, :],
                                     sup_T[:, n0:n1])
                    tmp = wkp.tile([128, 512], f32, tag="tmp")
                    nc.vector.scalar_tensor_tensor(
                        tmp[:, 0:n1 - n0], zs[:, 0:n1 - n0],
                        beta / (1.0 - beta), sup_T[:, n0:n1],
                        OP.mult, OP.add)
                    nc.scalar.activation(x_T[:, n0:n1], tmp[:, 0:n1 - n0],
                                         ACTF.Relu, scale=1.0 - beta)

                if l < L:
                    export_x()

            # ---- fc1: out = x8 @ w_fc1 + b_fc1, node-major
            ostage = pp.tile([128, NW, C], f32)
            osb = wkp.tile([C, 512], f32, tag="osb")
            for i in range(13):
                n0 = i * 512
                n1 = min(SHP, n0 + 512)
                op_ = psz.tile([C, 512], f32, tag="z")
                nc.tensor.matmul(op_[:, 0:n1 - n0], wfc1_t[:], x_T[:, n0:n1])
                nc.vector.tensor_scalar_add(osb[:, 0:n1 - n0], op_[:, 0:n1 - n0],
                                            bfc1_t[:])
                for k in range(n0 // 128, n1 // 128):
                    tp = pst.tile([128, C], f32, tag="tp")
                    nc.tensor.matmul(tp[:], osb[:, k * 128 - n0:(k + 1) * 128 - n0],
                                     ident40[:], is_transpose=True)
                    nc.scalar.copy(ostage[:, k, :], tp[:])
            nc.sync.dma_start(out_ext.ap().rearrange("(i p) c -> p i c", p=128),
                              ostage[:])

    nc.compile()
    _CACHE[key] = nc
    return nc


def kernel(**inputs):
    feats = np.asarray(inputs["features"], dtype=np.float32)
    edge_index = np.asarray(inputs["edge_index"])
    norm_A = np.asarray(inputs["norm_A"], dtype=np.float32)

    per_core, meta = _preprocess(edge_index, norm_A)
    nc = _build(meta)

    ident = np.eye(128, dtype=_BF16)
    ident40 = np.eye(C, dtype=np.float32)
    featp = np.zeros((SHP, IN), dtype=np.float32)

    in_maps = []
    for r in range(R):
        fp = np.zeros((SHP, IN), dtype=np.float32)
        fp[:SH] = feats[r * SH:(r + 1) * SH]
        in_maps.append({
            "features": fp,
            "w_fc0": np.asarray(inputs["w_fc0"], dtype=np.float32),
            "b_fc0": np.asarray(inputs["b_fc0"], dtype=np.float32),
            "conv_w": np.asarray(inputs["conv_w"], dtype=np.float32),
            "w_fc1": np.asarray(inputs["w_fc1"], dtype=np.float32),
            "b_fc1": np.asarray(inputs["b_fc1"], dtype=np.float32),
            "idx": per_core[r]["idx"],
            "S": per_core[r]["S"],
            "identb": ident,
            "ident40": ident40,
        })

    from concourse.bass_utils import run_bass_kernel_spmd
    trace = bool(os.environ.get("GCN_TRACE"))
    res = run_bass_kernel_spmd(nc, in_maps, list(range(R)), trace=trace)
    if trace:
        kernel.last_exec_time_ns = res.exec_time_ns
    out = np.concatenate([res.results[r]["out"][:SH] for r in range(R)], axis=0)
    return out.astype(np.float32)
